# revision 2
# baseline (speedup 1.0000x reference)
"""Multi-head causal attention (B=2, S=2048, D=1024, H=16) on 8 trn2 NeuronCores.

Strategy (tensor-parallel over heads, per the sharding hint):
  - Each core owns 2 heads (128 of 1024 hidden dims): W_q/W_k/W_v column-parallel.
  - Activations kept transposed ([dim, token]) end to end so every matmul
    contracts on the partition axis with zero on-device transposes of x.
  - Per batch: project all 4 token tiles, then run attention q-tiles
    HEAVY-FIRST (j=3,2,1,0) so the last q-tile before each collective is the
    cheapest one; batch-1 projections interleave with batch-0 attention.
  - scores^T = K^T.T @ Q^T per 128-key-chunk x 512-query-tile, two heads packed
    into disjoint PE row-groups (contraction is only dk=64) - they run
    concurrently in the array.
  - softmax without max-subtraction (scores are O(1)); rowsum folded into the
    PV matmul via an augmented V [keys, 64+1] whose last column is ones.
  - exp only on the causal part of diagonal chunks; the rest of the P tile is
    zeroed, and only the 128-wide diagonal strip is tri-masked.
  - reciprocals batched into one tiny [128, 8] DVE op per q-tile; row broadcast
    on the otherwise-idle GpSimd engine.
  - ctx re-sharded token-parallel with FOUR small AllToAlls (one per pair of
    q-tiles per batch), each fired the moment its 1024 tokens are ready, so
    out-projection for early pairs overlaps the remaining attention and the
    final collective only carries the two cheapest q-tiles.
  - out-projection per 128-token group with full W_o on each core.
  - bf16 matmul inputs everywhere; PSUM accumulation and softmax
    normalization stay fp32.

kernel(**inputs) takes the full unsharded inputs and returns the full output.
"""

import numpy as np
import ml_dtypes

import concourse.bass as bass
import concourse.mybir as mybir
import concourse.tile as tile
from concourse import bacc
from concourse.bass import ts
from concourse.bass_utils import run_bass_kernel_spmd
from concourse.tile_rust import add_dep_helper

B, S, D = 2, 2048, 1024
H, DK = 16, 64
NCORE = 8
T = B * S          # 4096 tokens
TT = 512           # token tile (projections, q-tiles)
NT = T // TT       # 8
KC = 128           # key chunk
NJ = S // TT       # 4 q-tiles per batch
PG = 128           # a2a token group (per dst core, per pair)
SCALE = 1.0 / np.sqrt(DK)

f32 = mybir.dt.float32
bf16 = mybir.dt.bfloat16
EXP = mybir.ActivationFunctionType.Exp
MULT = mybir.AluOpType.mult
npbf = ml_dtypes.bfloat16


def build_program():
    nc = bacc.Bacc("TRN2", target_bir_lowering=False, debug=False,
                   num_devices=NCORE)

    xT_d = nc.dram_tensor("xT", [NT, 128, 8, TT], bf16, kind="ExternalInput").ap()
    wT_d = nc.dram_tensor("wT", [128, 8, 3, 128], bf16, kind="ExternalInput").ap()
    woT_d = nc.dram_tensor("woT", [128, 8, 1024], bf16, kind="ExternalInput").ap()
    bqkv_d = nc.dram_tensor("bqkv", [128, 3], f32, kind="ExternalInput").ap()
    bo_d = nc.dram_tensor("bo", [128, 1024], f32, kind="ExternalInput").ap()
    trimask_d = nc.dram_tensor("trimask", [128, 128], bf16, kind="ExternalInput").ap()
    ident_d = nc.dram_tensor("ident", [128, 128], bf16, kind="ExternalInput").ap()
    # outT_d[b, p, t, od] = out token 1024*(1-p) + 128*core + t of batch b
    outT_d = nc.dram_tensor("outT", [B, 2, 128, 1024], f32, kind="ExternalOutput").ap()

    with tile.TileContext(nc) as tc:
        with (
            tc.tile_pool(name="const", bufs=1) as constp,
            tc.tile_pool(name="wostream", bufs=1) as wop,
            tc.tile_pool(name="xstream", bufs=3) as xp,
            tc.tile_pool(name="qkv", bufs=NT) as qkvp,
            tc.tile_pool(name="vaug", bufs=NJ) as vaugp,
            tc.tile_pool(name="ptile", bufs=4) as pp,
            tc.tile_pool(name="post", bufs=2) as postp,
            tc.tile_pool(name="cxn", bufs=4) as cxnp,
            tc.tile_pool(name="cxhold", bufs=4) as cxp,
            tc.tile_pool(name="outsb", bufs=2) as outp,
            tc.tile_pool(name="ps_s", bufs=2, space="PSUM") as ps_s,
            tc.tile_pool(name="ps_ctx", bufs=1, space="PSUM") as ps_ctx,
            tc.tile_pool(name="ps_misc", bufs=2, space="PSUM") as ps_misc,
            tc.tile_pool(name="dram", bufs=1, space="DRAM") as dramp,
        ):
            # ---- constants (wT loaded per-o inside proj_tile(0)) ----
            wT = constp.tile([128, 8, 3, 128], bf16, tag="wT")
            bqkv = constp.tile([128, 3], f32, tag="bqkv")
            trimask = constp.tile([128, 128], bf16, tag="trimask")
            ident = constp.tile([128, 128], bf16, tag="ident")

            # per-token-tile Q/K/V (transposed) and per-tile augmented V
            qkv_t = [[None] * NT for _ in range(3)]   # [j][t] -> [128, TT]
            vaug_t = [[[None] * NJ for _ in range(2)] for _ in range(B)]

            # one A2A per (batch, pair of q-tiles):
            # pair p=0 covers q-tiles {3,2} (tokens [1024,2048)), p=1 covers
            # {1,0} (tokens [0,1024)); dst core c takes 128 tokens at offset
            # 128c within the pair's 1024-token range.
            a2a_in = {(b, p): dramp.tile([NCORE, 128, PG], bf16,
                                         name=f"a2a_in{b}{p}")
                      for b in range(B) for p in range(2)}
            a2a_out = {(b, p): dramp.tile([NCORE, 128, PG], bf16,
                                          name=f"a2a_out{b}{p}")
                       for b in range(B) for p in range(2)}

            last_pair_dma = {}   # (b, p) -> last a2a_in write of that pair

            def proj_tile(t):
                xt = xp.tile([128, 8, TT], bf16, tag="xt")
                if t == 0:
                    # interleave the weight and x slices so the opening
                    # matmul starts after ~200KB instead of ~4MB
                    for o in range(8):
                        nc.sync.dma_start(wT[:, o], wT_d[:, o])
                        nc.sync.dma_start(xt[:, o, :], xT_d[t, :, o, :])
                    nc.sync.dma_start(bqkv[:], bqkv_d)
                    nc.sync.dma_start(ident[:], ident_d)
                else:
                    nc.sync.dma_start(xt[:], xT_d[t])
                for j in range(3):
                    ps = ps_misc.tile([128, TT], f32, tag="mm")
                    for o in range(8):
                        nc.tensor.matmul(ps[:], wT[:, o, j, :], xt[:, o, :],
                                         start=(o == 0), stop=(o == 7))
                    qt = qkvp.tile([128, TT], bf16, tag=f"qkv{j}",
                                   name=f"qkv{j}_{t}")
                    nc.vector.tensor_scalar_add(qt[:], ps[:], bqkv[:, j:j + 1])
                    qkv_t[j][t] = qt

            def vtrans_tile(t):
                b, tl = t // NJ, t % NJ
                va = [vaugp.tile([128, NJ, DK + 1], bf16, tag=f"va{b}{h}",
                                 name=f"va{b}{h}_{tl}") for h in range(2)]
                for h in range(2):
                    nc.vector.memset(va[h][:, :, DK:DK + 1], 1.0)
                    vaug_t[b][h][tl] = va[h]
                for kt in range(NJ):
                    ps_t = ps_misc.tile([128, TT], bf16, tag="mm")
                    nc.tensor.transpose(ps_t[:, 0:128],
                                        qkv_t[2][t][:, kt * KC:(kt + 1) * KC],
                                        ident[:])
                    for h in range(2):
                        nc.vector.tensor_copy(va[h][:, kt, 0:DK],
                                              ps_t[:, DK * h:DK * h + DK])

            def attention_qtile(b, j):
                nk = 4 * (j + 1)
                pc = [ps_ctx.tile([DK + 1, TT], f32, tag=f"c{h}", name=f"pc{h}")
                      for h in range(2)]

                def emit_pv(p_tile, m):
                    for h in range(2):
                        nc.tensor.matmul(
                            pc[h][:], vaug_t[b][h][m // 4][:, m % 4, :],
                            p_tile[:, TT * h:TT * (h + 1)],
                            start=(m == 0), stop=(m == nk - 1),
                            skip_group_check=True)

                qt = qkv_t[0][b * NJ + j]
                pending = []
                for m in range(nk):
                    kt_tile = qkv_t[1][b * NJ + m // 4]
                    ko = (m % 4) * KC
                    ps = ps_s.tile([128, 2 * TT], f32, tag="s")
                    nc.tensor.matmul(ps[:, 0:TT], kt_tile[0:DK, ko:ko + KC],
                                     qt[0:DK, :],
                                     start=True, stop=True, tile_position=(0, 0))
                    nc.tensor.matmul(ps[:, TT:], kt_tile[DK:128, ko:ko + KC],
                                     qt[DK:128, :],
                                     start=True, stop=True, tile_position=(64, 0))
                    p = pp.tile([128, 2 * TT], bf16, tag="p")
                    r = m - 4 * j
                    if r >= 0:
                        if r > 0:
                            nc.vector.memset(
                                p[:].rearrange("k (h q) -> k h q", h=2)[:, :, 0:KC * r],
                                0.0)
                        nc.scalar.activation(
                            p[:].rearrange("k (h q) -> k h q", h=2)[:, :, KC * r:],
                            ps[:].rearrange("k (h q) -> k h q", h=2)[:, :, KC * r:],
                            EXP, scale=float(SCALE))
                        nc.vector.tensor_tensor(
                            p[:].rearrange("k (h q) -> k h q", h=2)[:, :, KC * r:KC * (r + 1)],
                            p[:].rearrange("k (h q) -> k h q", h=2)[:, :, KC * r:KC * (r + 1)],
                            trimask[:, None, :].to_broadcast([128, 2, 128]), MULT)
                    else:
                        nc.scalar.activation(p[:], ps[:], EXP, scale=float(SCALE))
                    pending.append((p, m))
                    if len(pending) > 2:   # depth-2: PE never waits on a fresh exp
                        emit_pv(*pending.pop(0))
                for pm in pending:
                    emit_pv(*pm)

                # per-q-tile softmax normalization + ship to the A2A buffer.
                # cx/rtmp copies come first so the ctx PSUM banks free up
                # before the DVE queue hits the DMA-gated reciprocal; the tiny
                # gather DMAs ride the idle GpSimd SWDGE channel instead of
                # queueing behind megabyte x-tile loads on Sync.
                p_pair = (3 - j) // 2
                dst0 = 4 * (j - (2 - 2 * p_pair))   # high tile of pair -> 4..7
                rs_g = postp.tile([128, 8], f32, tag="rsg")
                cxs = []
                for h in range(2):
                    rtmp = cxnp.tile([1, TT], f32, tag="rtmp")
                    nc.vector.tensor_copy(rtmp[:], pc[h][DK:DK + 1, :])
                    cx = cxp.tile([DK, TT], f32, tag="cx")
                    nc.vector.tensor_copy(cx[:], pc[h][0:DK, :])
                    cxs.append(cx)
                    nc.gpsimd.dma_start(rs_g[:, 4 * h:4 * h + 4], rtmp[:])
                rc_g = postp.tile([128, 8], f32, tag="rcg")
                with nc.allow_low_precision(reason="softmax denominator"):
                    nc.vector.reciprocal(rc_g[:], rs_g[:])
                for h in range(2):
                    cx = cxs[h]
                    rrow = cxnp.tile([1, TT], f32, tag="rrow")
                    nc.gpsimd.dma_start(rrow[:], rc_g[:, 4 * h:4 * h + 4])
                    bcast = cxnp.tile([DK, TT], f32, tag="bcast")
                    nc.gpsimd.partition_broadcast(bcast[:], rrow[:], channels=DK)
                    cxn = cxnp.tile([DK, TT], bf16, tag="cxn")
                    nc.vector.tensor_tensor(cxn[:], cx[:], bcast[:], MULT)
                    for g in range(4):   # 128-token groups -> dst cores dst0+g
                        dma = nc.sync.dma_start(
                            a2a_in[(b, p_pair)][dst0 + g, DK * h:DK * (h + 1), :],
                            cxn[:, PG * g:PG * (g + 1)])
                        last_pair_dma[(b, p_pair)] = dma

            def do_a2a(b, p):
                nc.gpsimd.collective_compute(
                    "AllToAll", mybir.AluOpType.bypass,
                    replica_groups=[list(range(NCORE))],
                    ins=[a2a_in[(b, p)][:].opt()], outs=[a2a_out[(b, p)][:].opt()])

            ctx_tiles = {}

            def load_ctx(b, p, anchor):
                ctx_sb = constp.tile([128, 8, PG], bf16, tag=f"ctx{b}{p}",
                                     name=f"ctx{b}{p}")
                # one DMA per source rank; gate behind the given chain
                # anchor so the scheduler can't hoist the collective wait
                # ahead of attention-critical DMAs on the same queue.
                for d in range(8):
                    dma = nc.sync.dma_start(ctx_sb[:, d, :], a2a_out[(b, p)][d])
                    if anchor is not None:
                        add_dep_helper(dma.ins, anchor.ins, sync=False,
                                       reason="don't hoist a2a-gated ctx DMA")
                ctx_tiles[(b, p)] = ctx_sb

            def outproj(b, p):
                ctx_sb = ctx_tiles[(b, p)]
                # natural orientation: out[tok, od] = ctx_chunk.T @ woT_chunk
                for oh in range(2):      # 512-wide od halves
                    ps = ps_misc.tile([128, TT], f32, tag="mm")
                    for d in range(8):
                        nc.tensor.matmul(
                            ps[:], ctx_sb[:, d, :],
                            wo_sb[:, d, TT * oh:TT * (oh + 1)],
                            start=(d == 0), stop=(d == 7))
                    ot = outp.tile([128, TT], f32, tag="ot")
                    nc.vector.tensor_tensor(
                        ot[:], ps[:], bo_sb[:, TT * oh:TT * (oh + 1)],
                        mybir.AluOpType.add)
                    nc.sync.dma_start(
                        outT_d[b, p, :, TT * oh:TT * (oh + 1)], ot[:])

            # ---- schedule ----
            # batch-0 projections first (heavy-first attention needs all K/V)
            proj_tile(0)
            vtrans_tile(0)
            proj_tile(1)
            vtrans_tile(1)
            nc.sync.dma_start(trimask[:], trimask_d)
            proj_tile(2)
            vtrans_tile(2)
            proj_tile(3)
            vtrans_tile(3)
            # W_o / b_o arrive on the SWDGE channel during batch-0 attention
            wo_sb = wop.tile([128, 8, 1024], bf16, tag="wo")
            nc.gpsimd.dma_start(wo_sb[:], woT_d)
            bo_sb = wop.tile([128, 1024], f32, tag="bobc")
            nc.gpsimd.dma_start(bo_sb[:], bo_d)

            attention_qtile(0, 3)
            proj_tile(4)
            vtrans_tile(4)
            attention_qtile(0, 2)
            do_a2a(0, 0)
            proj_tile(5)
            vtrans_tile(5)
            attention_qtile(0, 1)
            proj_tile(6)
            vtrans_tile(6)
            attention_qtile(0, 0)
            do_a2a(0, 1)
            proj_tile(7)
            vtrans_tile(7)
            # ctx(0,0) anchors behind pair (0,1)'s last a2a_in write so its
            # gated descriptors never sit ahead of batch-0 attention DMAs.
            load_ctx(0, 0, last_pair_dma[(0, 1)])
            outproj(0, 0)
            attention_qtile(1, 3)
            load_ctx(0, 1, last_pair_dma[(0, 1)])
            outproj(0, 1)
            attention_qtile(1, 2)
            do_a2a(1, 0)
            attention_qtile(1, 1)
            attention_qtile(1, 0)
            do_a2a(1, 1)
            load_ctx(1, 0, last_pair_dma[(1, 1)])
            outproj(1, 0)
            load_ctx(1, 1, last_pair_dma[(1, 1)])
            outproj(1, 1)

    nc.compile()
    return nc


def make_in_maps(x, Wq, bq, Wk, bk, Wv, bv, Wo, bo):
    x = np.asarray(x, np.float32)
    xT = np.ascontiguousarray(x.reshape(T, D).T)                  # [D, T]
    # [NT, 128, 8, TT]: xT_t[t, p, o, q] = xT[o*128+p, t*TT+q]
    xT_t = np.ascontiguousarray(
        xT.reshape(8, 128, NT, TT).transpose(2, 1, 0, 3)).astype(npbf)

    woT = np.ascontiguousarray(
        np.asarray(Wo, np.float32).T.reshape(8, 128, 1024)
        .transpose(1, 0, 2)).astype(npbf)
    bo_bc = np.ascontiguousarray(
        np.broadcast_to(np.asarray(bo, np.float32)[None, :], (128, 1024)))

    trimask = (np.arange(128)[:, None] <= np.arange(128)[None, :]).astype(npbf)
    ident = np.eye(128, dtype=npbf)

    in_maps = []
    for c in range(NCORE):
        sl = slice(128 * c, 128 * (c + 1))
        wT_c = np.stack(
            [np.ascontiguousarray(
                np.asarray(W, np.float32)[sl, :].T.reshape(8, 128, 128)
                .transpose(1, 0, 2))
             for W in (Wq, Wk, Wv)], axis=2)                       # [128, 8, 3, 128]
        bqkv_c = np.stack([np.asarray(b_, np.float32)[sl]
                           for b_ in (bq, bk, bv)], axis=1)        # [128, 3]
        in_maps.append({
            "xT": xT_t,
            "wT": np.ascontiguousarray(wT_c).astype(npbf),
            "woT": woT,
            "bqkv": np.ascontiguousarray(bqkv_c),
            "bo": bo_bc,
            "trimask": trimask,
            "ident": ident,
        })
    return in_maps


def assemble_output(results):
    # results[c]["outT"]: [B, 2, 128, 1024]; pair p holds tokens
    # 1024*(1-p) + 128*c .. +128 of each batch
    out = np.empty((B, S, D), np.float32)
    for c in range(NCORE):
        r = results[c]["outT"]
        for p in range(2):
            base = 1024 * (1 - p) + PG * c
            out[:, base:base + PG, :] = r[:, p]
    return out


_PROGRAM = None


def get_program():
    global _PROGRAM
    if _PROGRAM is None:
        _PROGRAM = build_program()
    return _PROGRAM


def run(in_maps, **kwargs):
    nc = get_program()
    return run_bass_kernel_spmd(nc, in_maps, core_ids=list(range(NCORE)), **kwargs)


def kernel(x, Wq, bq, Wk, bk, Wv, bv, Wo, bo):
    in_maps = make_in_maps(x, Wq, bq, Wk, bk, Wv, bv, Wo, bo)
    res = run(in_maps)
    return assemble_output(res.results)


if __name__ == "__main__":
    rng = np.random.default_rng(0)
    x = rng.standard_normal((B, S, D), dtype=np.float32)
    mk = lambda *s: ((rng.random(s).astype(np.float32)) - 0.5) / 16
    out = kernel(x, mk(D, D), mk(D), mk(D, D), mk(D), mk(D, D), mk(D),
                 mk(D, D), mk(D))
    print(out.shape, out.dtype, np.abs(out).mean())
